# revision 1
# baseline (speedup 1.0000x reference)
"""Locally-connected 2D layer on 8 Trainium2 NeuronCores.

Problem: x[128,3,64,64] f32, per-position weights W[60,60,32,75], bias b[60,60,32]
  out[b,o,y,x] = sum_k patches[b,y,x,k] * W[y,x,o,k] + b[y,x,o],  k=(c,dy,dx)

Strategy (spatial sharding over output rows, 8 rows/core, memory-regime):
  - The contraction (c,dy,dx)=75 must live on SBUF partitions for the PE. dy is
    handled with a mod-5 ring of "patch planes" XP[(r%5, c, dx), x*128+b]; the
    per-row dy rotation is folded into the HOST-side W layout (np.roll), so the
    device always reads XP[0:76] as one contiguous partition range.
  - Ring planes are pre-replicated on the HOST (dx-im2col) into xpr[12,15,FXB]
    so every device fill is a plain [15, 30KB] DRAM->SBUF slice copy; fills are
    split into 4 free-chunks gated on the matmul chunks that last read the
    slot, so the ring advance overlaps the row's own compute.
  - Bias is folded in as contraction row 75 (W row 75 = bias, XP row 75 = 1.0).
  - Per output row: 15 groups of 4 column-tiled matmuls (lhsT=W[76,32],
    rhs=XP[76,128] -> out[32o,128b] at PSUM partitions 32j), PSUM->SBUF via DVE,
    one 983KB store per row in a DMA-friendly layout; host re-transposes once.
"""

import numpy as np

B, C, H, WIDTH = 128, 3, 64, 64
KH = KW = 5
RY = RX = 60
O = 32
K = 75
NCORES = 8
RPC = 8             # output rows computed per core (8*8=64, last 4 dropped)
INR = RPC + KH - 1  # 12 input rows per core
PADH = NCORES * RPC + KH - 1  # 68
NG = 15             # groups of 4 x-positions per row
CHUNKS = ((0, 4), (4, 4), (8, 4), (12, 3))  # (first group, n groups) per PSUM chunk
FXB = RX * B        # 7680 elements per patch plane

_cache = {}

USE_BF16 = True  # inputs (x-planes, W) in bf16; accumulation + output stay f32


def _build():
    import concourse.bass as bass
    import concourse.bacc as bacc
    import concourse.tile as tile
    import concourse.mybir as mybir

    f32 = mybir.dt.float32
    din = mybir.dt.bfloat16 if USE_BF16 else f32
    nc = bacc.Bacc("TRN2", target_bir_lowering=False, debug=False,
                   num_devices=NCORES)
    xpr_d = nc.dram_tensor("xpr", [INR, KH * C, FXB], din, kind="ExternalInput")
    wh_d = nc.dram_tensor("wh", [RPC, K + 1, RX, O], din, kind="ExternalInput")
    ones_d = nc.dram_tensor("ones", [1, FXB], din, kind="ExternalInput")
    oc_d = nc.dram_tensor("oc", [RPC, 4, O, NG, B], f32, kind="ExternalOutput")

    NPL = KH * C  # 15 planes per input row

    with tile.TileContext(nc) as tc:
        with (
            tc.tile_pool(name="const", bufs=1) as cpool,
            tc.tile_pool(name="w", bufs=4) as wpool,
            tc.tile_pool(name="os", bufs=2) as opool,
            tc.tile_pool(name="ps", bufs=4, space=bass.MemorySpace.PSUM) as ppool,
        ):
            xp = cpool.tile([K + 1, FXB], din)  # [76, 7680]; row 75 = ones

            nc.sync.dma_start(xp[K:K + 1, :], ones_d[:])
            for r in range(KH):  # initial ring: rows 0..4 -> slots 0..4
                nc.gpsimd.dma_start(xp[r * NPL:(r + 1) * NPL, :], xpr_d[r])

            wts = {}

            def load_w(k):
                wts[k] = wpool.tile([K + 1, RX * O], din, name="wt", tag="wt")
                nc.gpsimd.dma_start(wts[k][:],
                                    wh_d[k].rearrange("k x o -> k (x o)"))

            load_w(0)
            load_w(1)

            for k in range(RPC):
                wt = wts.pop(k)
                ot = opool.tile([128, NG * B], f32)  # [128, 1920]
                for ci, (g0, gn) in enumerate(CHUNKS):
                    pt = ppool.tile([128, 4 * B], f32)
                    for gs in range(gn):
                        for j in range(4):
                            xpos = (g0 + gs) * 4 + j
                            nc.tensor.matmul(
                                pt[32 * j:32 * (j + 1), gs * B:(gs + 1) * B],
                                wt[:, xpos * O:(xpos + 1) * O],
                                xp[:, xpos * B:(xpos + 1) * B],
                                tile_position=(0, 32 * j),
                            )
                    nc.vector.tensor_copy(
                        ot[:, g0 * B:(g0 + gn) * B], pt[:, :gn * B])
                    if k + KH < INR:
                        # ring advance for row k+1: overwrite slot k%5 with
                        # input row k+5, chunk-gated on this chunk's matmuls
                        slot = k % KH
                        f0, f1 = g0 * 4 * B, (g0 + gn) * 4 * B
                        nc.gpsimd.dma_start(
                            xp[slot * NPL:(slot + 1) * NPL, f0:f1],
                            xpr_d[k + KH, :, f0:f1])
                if k + 2 < RPC:
                    load_w(k + 2)
                nc.scalar.dma_start(
                    oc_d[k].rearrange("j o g b -> (j o) (g b)"), ot[:])

    nc.compile()
    return nc


def _get_nc():
    if "nc" not in _cache:
        _cache["nc"] = _build()
    return _cache["nc"]


def _prep_inputs(x, W, b):
    x = np.asarray(x, np.float32)
    W = np.asarray(W, np.float32)
    b = np.asarray(b, np.float32)
    xh = np.zeros((PADH, C, WIDTH, B), np.float32)
    xh[:H] = x.transpose(2, 1, 3, 0)  # [row, c, w, batch]
    # ring planes: xpr_full[r, (c,dx) -> c*KW+dx, x, b] = xh[r, c, x+dx, b]
    # plane order within a slot must be p2 = c*KW + dx (with slot-major rm)
    xpr_full = np.zeros((PADH, C, KW, RX, B), np.float32)
    for dx in range(KW):
        xpr_full[:, :, dx] = xh[:, :, dx:dx + RX]
    xpr_full = xpr_full.reshape(PADH, C * KW, FXB)
    Wfull = W.transpose(0, 3, 1, 2)  # [RY, K, RX, O]
    in_maps = []
    for i in range(NCORES):
        whc = np.zeros((RPC, K + 1, RX, O), np.float32)
        for k in range(RPC):
            y = RPC * i + k
            if y < RY:
                w5 = Wfull[y].reshape(C, KH, KW, RX, O)
                # device slot rm holds input row with (local row)%5 == rm;
                # slot rm supplies dy=(rm-k)%5 for output row k -> roll by k.
                # partition order: p = rm*15 + c*5 + dx
                whc[k, :K] = np.roll(w5, k, axis=1).transpose(1, 0, 2, 3, 4) \
                    .reshape(K, RX, O)
                whc[k, K] = b[y]
        if USE_BF16:
            import ml_dtypes
            bf = ml_dtypes.bfloat16
            in_maps.append({
                "xpr": np.ascontiguousarray(
                    xpr_full[RPC * i:RPC * i + INR]).astype(bf),
                "wh": whc.astype(bf),
                "ones": np.ones((1, FXB), bf),
            })
        else:
            in_maps.append({
                "xpr": np.ascontiguousarray(xpr_full[RPC * i:RPC * i + INR]),
                "wh": whc,
                "ones": np.ones((1, FXB), np.float32),
            })
    return in_maps


def kernel(x, W, b):
    from concourse.bass_utils import run_bass_kernel_spmd

    nc = _get_nc()
    in_maps = _prep_inputs(x, W, b)
    br = run_bass_kernel_spmd(nc, in_maps, list(range(NCORES)),
                              **_cache.get("run_kwargs", {}))
    _cache["last_run"] = br
    oc = np.stack([np.asarray(br.results[i]["oc"]) for i in range(NCORES)])
    oc = oc.reshape(NCORES * RPC, 4, O, NG, B)  # [64, j, o, x4, b]
    out = oc.transpose(4, 2, 0, 3, 1).reshape(B, O, NCORES * RPC, RX)
    return np.ascontiguousarray(out[:, :, :RY, :])



# revision 5
# speedup vs baseline: 1.2798x; 1.2798x over previous
"""Locally-connected 2D layer on 8 Trainium2 NeuronCores.

Problem: x[128,3,64,64] f32, per-position weights W[60,60,32,75], bias b[60,60,32]
  out[b,o,y,x] = sum_k patches[b,y,x,k] * W[y,x,o,k] + b[y,x,o],  k=(c,dy,dx)

Strategy (spatial sharding over output rows, 8 rows/core, memory-regime):
  - Contraction (c,dy,dx)=75 (+1 bias) on SBUF partitions. dy handled by a
    mod-5 ring of patch planes; the per-row dy rotation is folded into the
    host-side W layout so the device always reads partitions 0:76.
  - Partition interleave p = plane*5 + slot (plane=c*5+dx) so a slot refill
    writes partitions {s, s+5, ..., s+70} -> spread over ~11 DMA engines
    instead of 4 for a contiguous 15-partition block.
  - The x range is split in two halves, each its own SBUF tile (xpA: x<32,
    xpB: x>=32) so the two per-row refill DMAs hit different tensors and
    never need write-write ordering. Same for W (3 batched preloads into 3
    tiles) and the output row staging (otA drained by DVE, otB by ACT).
  - Initial ring fill = 2 big HWDGE DMAs; refills = 1 HWDGE DMA per half per
    row, gated on the matmul chunk that last read the slot. W preloads on
    the gpsimd (SWDGE) queue with multi-row batching.
  - Bias folded as contraction row 75 (W row 75 = bias, XP row 75 = 1.0).
  - Per output row: 2 PSUM chunks of [128, 1024] f32 (2 banks each); each
    chunk is col-tiled matmuls (lhsT=W[76,32], rhs=XP[76,128] -> [32o,128b]
    at PSUM partition 32j). Drains cast f32->bf16; 2 bf16 stores per row.
"""

import numpy as np

B, C, H, WIDTH = 128, 3, 64, 64
KH = KW = 5
RY = RX = 60
O = 32
K = 75
NCORES = 8
RPC = 8             # output rows computed per core (8*8=64, last 4 dropped)
INR = RPC + KH - 1  # 12 input rows per core
PADH = NCORES * RPC + KH - 1  # 68
FXB = RX * B        # 7680 elements per patch plane
NPL = KH * C        # 15 planes per input row
GA = 8              # groups (of 4 x-positions) in chunk A
FA = GA * 4 * B     # 4096 patch-plane elems covered by chunk A (x < 32)
GB = 15 - GA        # groups in chunk B
FB = FXB - FA       # 3584 elems in chunk B (x >= 32)
WSPLITS = ((0, 1), (1, 3), (3, 8))  # W preload batches (k0, k1)

_cache = {}


def _build():
    import concourse.bass as bass
    import concourse.bacc as bacc
    import concourse.tile as tile
    import concourse.mybir as mybir

    f32 = mybir.dt.float32
    bf16 = mybir.dt.bfloat16
    nc = bacc.Bacc("TRN2", target_bir_lowering=False, debug=False,
                   num_devices=NCORES)
    xpr_d = nc.dram_tensor("xpr", [INR, NPL, FXB], bf16, kind="ExternalInput")
    wh_d = nc.dram_tensor("wh", [RPC, K + 1, RX, O], bf16, kind="ExternalInput")
    ones_d = nc.dram_tensor("ones", [1, FXB], bf16, kind="ExternalInput")
    oc_d = nc.dram_tensor("oc", [RPC, 4, O, 15, B], bf16, kind="ExternalOutput")

    with tile.TileContext(nc) as tc:
        with (
            tc.tile_pool(name="const", bufs=1) as cpool,
            tc.tile_pool(name="os", bufs=2) as opool,
            tc.tile_pool(name="ps", bufs=3, space=bass.MemorySpace.PSUM) as ppool,
        ):
            xpA = cpool.tile([K + 1, FA], bf16)   # x 0..31; row 75 = ones
            xpB = cpool.tile([K + 1, FB], bf16)   # x 32..59; row 75 = ones
            wts = [cpool.tile([K + 1, (k1 - k0) * RX * O], bf16,
                              name=f"wt{k0}")
                   for k0, k1 in WSPLITS]

            nc.sync.dma_start(xpA[K:K + 1, :], ones_d[:, :FA])
            nc.sync.dma_start(xpB[K:K + 1, :], ones_d[:, FA:])
            # initial ring fill: src iterates (plane, row, f) to match the
            # interleaved partition order p = plane*KH + row
            nc.sync.dma_start(xpA[:K, :],
                              xpr_d[0:KH, :, :FA].transpose([1, 0, 2]))
            nc.sync.dma_start(xpB[:K, :],
                              xpr_d[0:KH, :, FA:].transpose([1, 0, 2]))

            for wi, (k0, k1) in enumerate(WSPLITS):
                nc.gpsimd.dma_start(
                    wts[wi][:, :].rearrange("p (k x o) -> p k x o",
                                            k=k1 - k0, x=RX, o=O),
                    wh_d[k0:k1].transpose([1, 0, 2, 3]))

            def wslice(k, x):
                # lhsT [76, 32] for output row k, x-position x
                for wi, (k0, k1) in enumerate(WSPLITS):
                    if k0 <= k < k1:
                        off = ((k - k0) * RX + x) * O
                        return wts[wi][:, off:off + O]

            for k in range(RPC):
                otA = opool.tile([128, GA * B], bf16, name="otA")
                otB = opool.tile([128, GB * B], bf16, name="otB")
                # chunk A: x = 0..31
                pa = ppool.tile([128, GA * B], f32, name="pt", tag="pt")
                for g in range(GA):
                    for j in range(4):
                        x = g * 4 + j
                        nc.tensor.matmul(
                            pa[32 * j:32 * (j + 1), g * B:(g + 1) * B],
                            wslice(k, x),
                            xpA[:, x * B:(x + 1) * B],
                            tile_position=(0, 32 * j),
                        )
                if k + KH < INR:
                    nc.sync.dma_start(xpA[k % KH:K:KH, :],
                                      xpr_d[k + KH, :, :FA])
                nc.vector.tensor_copy(otA[:], pa[:])
                # chunk B: x = 32..59
                pb = ppool.tile([128, GA * B], f32, name="pt", tag="pt")
                for g in range(GB):
                    for j in range(4):
                        x = (GA + g) * 4 + j
                        nc.tensor.matmul(
                            pb[32 * j:32 * (j + 1), g * B:(g + 1) * B],
                            wslice(k, x),
                            xpB[:, (x - 32) * B:(x - 31) * B],
                            tile_position=(0, 32 * j),
                        )
                if k + KH < INR:
                    nc.sync.dma_start(xpB[k % KH:K:KH, :],
                                      xpr_d[k + KH, :, FA:])
                nc.scalar.copy(otB[:], pb[:, :GB * B])
                nc.scalar.dma_start(
                    oc_d[k, :, :, :GA].rearrange("j o g b -> (j o) (g b)"),
                    otA[:])
                nc.scalar.dma_start(
                    oc_d[k, :, :, GA:].rearrange("j o g b -> (j o) (g b)"),
                    otB[:])

    nc.compile()
    return nc


def _get_nc():
    if "nc" not in _cache:
        _cache["nc"] = _build()
    return _cache["nc"]


def _prep_inputs(x, W, b):
    import ml_dtypes
    bf = ml_dtypes.bfloat16

    x = np.asarray(x, np.float32)
    W = np.asarray(W, np.float32)
    b = np.asarray(b, np.float32)
    xh = np.zeros((PADH, C, WIDTH, B), np.float32)
    xh[:H] = x.transpose(2, 1, 3, 0)  # [row, c, w, batch]
    # patch planes: xpr_full[r, plane = c*KW+dx, x*B+b] = xh[r, c, x+dx, b]
    xpr_full = np.zeros((PADH, C, KW, RX, B), np.float32)
    for dx in range(KW):
        xpr_full[:, :, dx] = xh[:, :, dx:dx + RX]
    xpr_full = xpr_full.reshape(PADH, NPL, FXB).astype(bf)

    # W partition map: (c, dy, dx) -> p = (c*KW+dx)*KH + (k+dy)%KH
    cidx = np.arange(C)[:, None, None]
    dyidx = np.arange(KH)[None, :, None]
    dxidx = np.arange(KW)[None, None, :]
    in_maps = []
    for i in range(NCORES):
        whc = np.zeros((RPC, K + 1, RX, O), np.float32)
        for k in range(RPC):
            y = RPC * i + k
            if y < RY:
                # W[y]: [RX, O, 75] with kidx = c*25 + dy*5 + dx
                wy = W[y].reshape(RX, O, C, KH, KW).transpose(2, 3, 4, 0, 1)
                pidx = ((cidx * KW + dxidx) * KH + (k + dyidx) % KH)
                whc[k, pidx.reshape(-1)] = wy.reshape(K, RX, O)
                whc[k, K] = b[y]
        in_maps.append({
            "xpr": np.ascontiguousarray(xpr_full[RPC * i:RPC * i + INR]),
            "wh": whc.astype(bf),
            "ones": np.ones((1, FXB), bf),
        })
    return in_maps


def kernel(x, W, b):
    from concourse.bass_utils import run_bass_kernel_spmd

    nc = _get_nc()
    in_maps = _prep_inputs(x, W, b)
    br = run_bass_kernel_spmd(nc, in_maps, list(range(NCORES)),
                              **_cache.get("run_kwargs", {}))
    _cache["last_run"] = br
    oc = np.stack([np.asarray(br.results[i]["oc"]).astype(np.float32)
                   for i in range(NCORES)])
    oc = oc.reshape(NCORES * RPC, 4, O, 15, B)  # [64, j, o, g, b]
    out = oc.transpose(4, 2, 0, 3, 1).reshape(B, O, NCORES * RPC, RX)
    return np.ascontiguousarray(out[:, :, :RY, :])


# revision 7
# speedup vs baseline: 1.4524x; 1.1348x over previous
"""Locally-connected 2D layer on 8 Trainium2 NeuronCores.

Problem: x[128,3,64,64] f32, per-position weights W[60,60,32,75], bias b[60,60,32]
  out[b,o,y,x] = sum_k patches[b,y,x,k] * W[y,x,o,k] + b[y,x,o],  k=(c,dy,dx)

Strategy (spatial sharding over output rows, 8 rows/core, memory-regime):
  - 4 output rows are computed per matmul ("super-row"): the mod-8 ring holds
    8 input rows as planes on partitions p = plane*8 + slot (plane=c*5+dx in
    [0,15), slot in [0,8)) -> 120 partitions, +1 ones row at p=120 for the
    bias. lhsT = W[121, 128] packs (k,o) for 4 rows; each k's weights sit on
    its own 75 active plane-partitions (zero elsewhere). One full-array
    [121,128]x[121,128] matmul per x-position -> out[(k,o), b] in PSUM.
    128-col weights enable the compiler's fast-weight-load path.
  - Per-row dy rotation is folded into the host-side W layout (slot =
    (row+dy) mod 8), so the device reads partitions 0:121 always.
  - The x range is split in two SBUF tiles (xpA: x<32, xpB: x>=32) so refill
    DMAs and matmul reads of different halves never need write-write
    ordering. Slot refills (input rows 8..11, between the two super-rows)
    write 15 stride-8 partitions -> spread over ~15 DMA engines.
  - All DMAs on the two HWDGE queues: sync = patch fills + refills,
    scalar = W preloads + output stores. Everything chunked along x so the
    first matmul starts after ~0.5MB of input instead of 4MB.
  - PSUM chunks of 8 x-positions ([128, 1024] f32, 2 banks); drains
    alternate DVE / ACT, casting f32->bf16; one store per chunk.
"""

import numpy as np

B, C, H, WIDTH = 128, 3, 64, 64
KH = KW = 5
RY = RX = 60
O = 32
K = 75
NCORES = 8
RPC = 8             # output rows per core (8*8=64, last 4 dropped)
SR = 2              # super-rows per core (4 output rows each)
INR = RPC + KH - 1  # 12 input rows per core
PADH = NCORES * RPC + KH - 1  # 68
FXB = RX * B        # 7680 elements per patch plane
NPL = KH * C        # 15 planes per input row
NSL = 8             # ring slots
KP = NPL * NSL + 1  # 121 contraction partitions (120 planes + ones)
XA = 32             # x-positions in tile A
FA = XA * B         # 4096
FB = FXB - FA       # 3584
CHUNKS = ((0, 8), (8, 16), (16, 24), (24, 32),
          (32, 40), (40, 48), (48, 56), (56, 60))

_cache = {}


def _build():
    import concourse.bass as bass
    import concourse.bacc as bacc
    import concourse.tile as tile
    import concourse.mybir as mybir

    f32 = mybir.dt.float32
    bf16 = mybir.dt.bfloat16
    nc = bacc.Bacc("TRN2", target_bir_lowering=False, debug=False,
                   num_devices=NCORES)
    xpr_d = nc.dram_tensor("xpr", [INR, NPL, FXB], bf16, kind="ExternalInput")
    # wh[sr, p, x, k, o]
    wh_d = nc.dram_tensor("wh", [SR, KP, RX, 4, O], bf16, kind="ExternalInput")
    ones_d = nc.dram_tensor("ones", [1, FXB], bf16, kind="ExternalInput")
    # oc[sr, (k,o), x, b]
    oc_d = nc.dram_tensor("oc", [SR, 4 * O, RX, B], bf16, kind="ExternalOutput")

    with tile.TileContext(nc) as tc:
        with (
            tc.tile_pool(name="const", bufs=1) as cpool,
            tc.tile_pool(name="os", bufs=4) as opool,
            tc.tile_pool(name="ps", bufs=3, space=bass.MemorySpace.PSUM) as ppool,
        ):
            xpA = cpool.tile([KP, FA], bf16)
            xpB = cpool.tile([KP, FB], bf16)
            wts = [cpool.tile([KP, RX * 4 * O], bf16, name=f"wt{sr}")
                   for sr in range(SR)]

            nc.sync.dma_start(xpA[KP - 1:KP, :], ones_d[:, :FA])
            nc.sync.dma_start(xpB[KP - 1:KP, :], ones_d[:, FA:])
            # initial ring fill (input rows 0..7), chunked along x; src
            # iterates (plane, row, f) to match p = plane*NSL + row
            for x0, x1 in CHUNKS[:4]:
                nc.sync.dma_start(
                    xpA[:KP - 1, x0 * B:x1 * B],
                    xpr_d[0:NSL, :, x0 * B:x1 * B].transpose([1, 0, 2]))
            for x0, x1 in ((32, 46), (46, 60)):
                nc.sync.dma_start(
                    xpB[:KP - 1, (x0 - XA) * B:(x1 - XA) * B],
                    xpr_d[0:NSL, :, x0 * B:x1 * B].transpose([1, 0, 2]))

            # W preloads, chunked along x (wt0 finer: it gates startup)
            for x0, x1 in ((0, 15), (15, 30), (30, 45), (45, 60)):
                nc.scalar.dma_start(wts[0][:, x0 * 128:x1 * 128],
                                    wh_d[0, :, x0:x1])
            for x0, x1 in ((0, 30), (30, 60)):
                nc.scalar.dma_start(wts[1][:, x0 * 128:x1 * 128],
                                    wh_d[1, :, x0:x1])

            for sr in range(SR):
                for ci, (x0, x1) in enumerate(CHUNKS):
                    nx = x1 - x0
                    pt = ppool.tile([128, 8 * B], f32, name="pt", tag="pt")
                    for xi in range(nx):
                        x = x0 + xi
                        src = (xpA[:, x * B:(x + 1) * B] if x < XA else
                               xpB[:, (x - XA) * B:(x - XA + 1) * B])
                        nc.tensor.matmul(
                            pt[:, xi * B:(xi + 1) * B],
                            wts[sr][:, x * 128:(x + 1) * 128],
                            src,
                            tile_position=(0, 0),
                        )
                    if sr == 0 and x1 == XA:
                        for s in range(INR - NSL):
                            nc.sync.dma_start(xpA[s:KP - 1:NSL, :],
                                              xpr_d[NSL + s, :, :FA])
                    if sr == 0 and x1 == RX:
                        for s in range(INR - NSL):
                            nc.sync.dma_start(xpB[s:KP - 1:NSL, :],
                                              xpr_d[NSL + s, :, FA:])
                    ot = opool.tile([128, 8 * B], bf16, name="ot")
                    if ci % 2 == 0:
                        nc.vector.tensor_copy(ot[:, :nx * B], pt[:, :nx * B])
                    else:
                        nc.scalar.copy(ot[:, :nx * B], pt[:, :nx * B])
                    nc.scalar.dma_start(oc_d[sr, :, x0:x1, :].rearrange(
                        "p x b -> p (x b)"), ot[:, :nx * B])

    nc.compile()
    return nc


def _get_nc():
    if "nc" not in _cache:
        _cache["nc"] = _build()
    return _cache["nc"]


def _prep_inputs(x, W, b):
    import ml_dtypes
    bf = ml_dtypes.bfloat16

    x = np.asarray(x, np.float32)
    W = np.asarray(W, np.float32)
    b = np.asarray(b, np.float32)
    xh = np.zeros((PADH, C, WIDTH, B), np.float32)
    xh[:H] = x.transpose(2, 1, 3, 0)  # [row, c, w, batch]
    # patch planes: xpr_full[r, plane = c*KW+dx, x*B+b] = xh[r, c, x+dx, b]
    xpr_full = np.zeros((PADH, C, KW, RX, B), np.float32)
    for dx in range(KW):
        xpr_full[:, :, dx] = xh[:, :, dx:dx + RX]
    xpr_full = xpr_full.reshape(PADH, NPL, FXB).astype(bf)

    # W partition map: (c, dy, dx) -> p = (c*KW+dx)*NSL + (row+dy)%NSL
    cidx = np.arange(C)[:, None, None]
    dyidx = np.arange(KH)[None, :, None]
    dxidx = np.arange(KW)[None, None, :]
    in_maps = []
    for i in range(NCORES):
        whc = np.zeros((SR, KP, RX, 4, O), np.float32)
        for sr in range(SR):
            for k in range(4):
                r = sr * 4 + k
                y = RPC * i + r
                if y < RY:
                    # W[y]: [RX, O, 75] with kidx = c*25 + dy*5 + dx
                    wy = W[y].reshape(RX, O, C, KH, KW).transpose(2, 3, 4, 0, 1)
                    pidx = ((cidx * KW + dxidx) * NSL + (r + dyidx) % NSL)
                    whc[sr, pidx.reshape(-1), :, k, :] = wy.reshape(K, RX, O)
                    whc[sr, KP - 1, :, k, :] = b[y]
        in_maps.append({
            "xpr": np.ascontiguousarray(xpr_full[RPC * i:RPC * i + INR]),
            "wh": whc.astype(bf),
            "ones": np.ones((1, FXB), bf),
        })
    return in_maps


def kernel(x, W, b):
    from concourse.bass_utils import run_bass_kernel_spmd

    nc = _get_nc()
    in_maps = _prep_inputs(x, W, b)
    br = run_bass_kernel_spmd(nc, in_maps, list(range(NCORES)),
                              **_cache.get("run_kwargs", {}))
    _cache["last_run"] = br
    oc = np.stack([np.asarray(br.results[i]["oc"]).astype(np.float32)
                   for i in range(NCORES)])
    # oc: [core, sr, (k,o), x, b] -> out[b, o, y= core*8+sr*4+k, x]
    oc = oc.reshape(NCORES * SR, 4, O, RX, B)
    out = oc.transpose(4, 2, 0, 1, 3).reshape(B, O, NCORES * RPC, RX)
    return np.ascontiguousarray(out[:, :, :RY, :])
